# revision 3
# baseline (speedup 1.0000x reference)
"""Trainium2 Bass kernel for nn_CrossStockRelationship, v2.

Computation (reference):
    rel_encoded = MLP(relationship_matrix[stock_idx])      # [S, H], tiny
    rel_encoded[stock_idx] = 0                             # mask
    out[b, h]  = sum_s encoded_states[b, s, h] * rel_encoded[s, h]

Memory-bound. Device ships every enc element once as fp8 e3m4 (1 B)
-> per-core DMA floor 16.4 MB / 360 B/ns = 45.5 us. Engines split the
contraction so each stays under that window:

- h-PHASED stream: phase q delivers ALL stocks' h-range [16q,16q+16)
  (13 PE quarter-tiles + interleaved DVE half-tiles). Chunk q's PSUM
  accumulation closes at the phase boundary, so its evacuation overlaps
  phase q+1's matmuls; only chunk 3's evac is in the tail.
- PE: 13 blocks (12xK=124 + 1xK=128 = 1616 stocks) as per-h matvecs,
  stationary rel[:, h] bf16, moving enc[s, (h b)] e3m4, f32 PSUM.
  PSUM: chunk q -> partition 32*(q//2) (only 0/32 are HW-encodable),
  banks (q%2)*4..+3, so evac reads and next-phase writes touch
  different banks (the Tile hazard tracker is partition-blind).
  start=True zeroes a whole 2KB psum bank (zero region, 4 h-columns):
  it is issued only on each bank's first h, stop on its last —
  per-h start flags wipe neighbouring h's first-block contributions
  (measured 0.24 rel err, reproduced exactly in CoreSim).
- DVE: 3 blocks of 128 stocks in half-batch granules. ACT converts
  e3m4 -> bf16 (engine otherwise idle), DVE does bf16 mul (2x mode) +
  in-place binary tree reduce, f32 for the last two levels.
- Error: e3m4 quantization of enc gives 1.35e-2 (host-simulated);
  device measures 1.36e-2 total vs the 2e-2 gate. (The baseline's
  extra ~1.15e-2 "device floor" was the zero-region wipe; the per-bank
  start/stop protocol removes it.)
"""

import os
import sys

for _p in ("/opt/trn_rl_repo", "/root/.axon_site/_ro/trn_rl_repo"):
    if os.path.isdir(_p) and _p not in sys.path:
        sys.path.insert(0, _p)

import numpy as np
import ml_dtypes

import concourse.bass as bass
import concourse.bacc as bacc
import concourse.tile as tile
from concourse import mybir
from concourse.bass_utils import run_bass_kernel_spmd

N_CORES = 8
B = 1024
S = 2000
H = 64
BC = B // N_CORES  # 128 batches per core
BH = BC // 2  # 64 batches per DVE partition-half

# PE blocks: 12 x 124 + 1 x 128 = 1616 stocks
N_DVE = int(os.environ.get("KV2_NDVE", "3"))
K_DVE = 128
S_PE = S - N_DVE * K_DVE
_npe_full = S_PE // 124  # blocks of 124
_rem = S_PE - (_npe_full - 1) * 124 if S_PE % 124 else 0
if S_PE % 124 == 0:
    PE_KS = [124] * _npe_full
else:
    # last block absorbs the remainder (<= 128)
    PE_KS = [124] * (S_PE // 124)
    PE_KS[-1] += S_PE % 124
    assert PE_KS[-1] <= 128
N_PE = len(PE_KS)
PE_OFFS = [sum(PE_KS[:j]) for j in range(N_PE)]
assert sum(PE_KS) == S_PE

WARMUP = os.environ.get("KV2_WARMUP", "0") == "1"
WARMUP_N = int(os.environ.get("KV2_WARMUP_N", "24"))
MID_PARTS = int(os.environ.get("KV2_MID_PARTS", "4"))
LAST_PARTS = int(os.environ.get("KV2_LAST_PARTS", "4"))
EVAC_INLINE = os.environ.get("KV2_EVAC_INLINE", "1") == "1"
PSUM_ALT = os.environ.get("KV2_PSUM_ALT", "1") == "1"
# After which flat part index (phase*N_PE + j) each DVE half-tile streams.
DV_SLOTS = [int(x) for x in os.environ.get(
    "KV2_DV_SLOTS", "9,15,21,27,33,40").split(",")]
assert len(DV_SLOTS) == 2 * N_DVE

TRACE = False
LAST_RESULT = None
_NC_CACHE = {}


def _build(nc, tc, tensors, ctx):
    f32 = mybir.dt.float32
    bf16 = mybir.dt.bfloat16
    e3 = mybir.dt.float8e3

    rel_pool = ctx.enter_context(tc.tile_pool(name="rel", bufs=1))
    pe_pool = ctx.enter_context(tc.tile_pool(name="pe", bufs=4))
    pes_pool = ctx.enter_context(tc.tile_pool(name="pes", bufs=14))
    dv8_pool = ctx.enter_context(tc.tile_pool(name="dv8", bufs=2))
    dv16_pool = ctx.enter_context(tc.tile_pool(name="dv16", bufs=2))
    prod_pool = ctx.enter_context(tc.tile_pool(name="prod", bufs=2))
    fv_pool = ctx.enter_context(tc.tile_pool(name="fv", bufs=2))
    ov_pool = ctx.enter_context(tc.tile_pool(name="ov", bufs=2))
    out_pool = ctx.enter_context(tc.tile_pool(name="out", bufs=2))
    psum_pool = ctx.enter_context(tc.tile_pool(name="psum", bufs=1, space="PSUM"))
    if WARMUP:
        wu_pool = ctx.enter_context(tc.tile_pool(name="wu", bufs=1))

    # stationary weights (tiny, land early)
    rel16_t = rel_pool.tile([128, N_PE * H], bf16)
    nc.scalar.dma_start(out=rel16_t[:, :], in_=tensors["rel16"][:, :])
    rel_dve_t = rel_pool.tile([128, N_DVE * K_DVE], bf16)
    nc.scalar.dma_start(out=rel_dve_t[:, :], in_=tensors["rel_dve"][:, :])

    # PSUM: chunk q = h//16 -> partition 32*(q//2) (only offsets 0/32 are
    # HW-encodable), banks (q%2)*4..+3 via an 8KB column offset. Phase q's
    # evac reads banks disjoint from phase q+1's matmul writes, so the
    # boundary evac overlaps the next phase with no false hazard.
    ps = psum_pool.tile([64, 32 * BC], f32)

    def chunk_pos(q):
        return 32 * (q // 2)

    def chunk_cols(q):
        return (q % 2) * 16 * BC

    def mm(h, lhsT, rhs, first, last):
        pos = chunk_pos(h // 16)
        col = chunk_cols(h // 16) + (h % 16) * BC
        # start=True zeroes the whole 2KB psum bank (zero region), which
        # holds 4 h-columns: issue it only for the bank's first h — the
        # other three inherit the pending-zero (read-as-zero) state.
        # Symmetrically, stop only on the bank's last h.
        nc.tensor.matmul(
            out=ps[pos : pos + 1, col : col + BC],
            lhsT=lhsT,
            rhs=rhs,
            start=first and (h % 4 == 0),
            stop=last and (h % 4 == 3),
            # the group checker can't express a 4-subcolumn shared-bank
            # group; correctness is enforced by the start/stop protocol
            skip_group_check=True,
            tile_position=(0, pos),
        )

    if WARMUP:
        # Keep the PE busy (and its clock ramping) from t~0.3us until the
        # first real tile lands: dummy matmuls on a memset tile into a psum
        # region the first real (start=True) matmul resets anyway.
        wt = wu_pool.tile([128, 128], bf16)
        nc.gpsimd.memset(wt[:, :], 0.0)
        for _ in range(WARMUP_N):
            nc.tensor.matmul(
                out=ps[0:1, 0:BC],
                lhsT=wt[:, 0:1],
                rhs=wt[:, 0:BC],
                start=True,
                stop=True,
                skip_group_check=True,
                tile_position=(0, 0),
            )

    # DVE-block pipeline, in batch-half granules (BQ=32 batches x 128
    # stocks): DMA (sync queue, interleaved into the PE tile stream) ->
    # ACT fp8->bf16 convert -> DVE bf16 mul (2x mode) + in-place tree ->
    # out DMA (gpsimd/SWDGE, mid-kernel so its latency is hidden).
    BQ = BH // 2

    def emit_dve_half(g, half):
        et8 = dv8_pool.tile([128, BQ * K_DVE], e3, tag="dv8")
        nc.sync.dma_start(
            out=et8[:, :],
            in_=tensors[f"encdv{g}"][:, half * BQ * K_DVE : (half + 1) * BQ * K_DVE],
        )
        et16 = dv16_pool.tile([128, BQ * K_DVE], bf16, tag="dv16")
        nc.scalar.activation(
            out=et16[:, :],
            in_=et8[:, :],
            func=mybir.ActivationFunctionType.Copy,
            bias=0.0,
            scale=1.0,
        )
        pt = prod_pool.tile([128, BQ * K_DVE], bf16, tag="prod")
        rb = (
            rel_dve_t[:, g * K_DVE : (g + 1) * K_DVE]
            .rearrange("p (o s) -> p o s", o=1)
            .broadcast_to([128, BQ, K_DVE])
        )
        pv = pt[:, :].rearrange("p (b s) -> p b s", s=K_DVE)
        nc.vector.tensor_mul(
            pv, et16[:, :].rearrange("p (b s) -> p b s", s=K_DVE), rb
        )
        s_len = K_DVE
        while s_len > 4:
            nc.vector.tensor_add(
                pv[:, :, 0 : s_len // 2],
                pv[:, :, 0 : s_len // 2],
                pv[:, :, s_len // 2 : s_len],
            )
            s_len //= 2
        fv = fv_pool.tile([128, BQ * 2], f32, tag="fv")
        f2 = fv[:, :].rearrange("p (b s) -> p b s", s=2)
        nc.vector.tensor_add(f2[:, :, :], pv[:, :, 0:2], pv[:, :, 2:4])
        ov = ov_pool.tile([128, BQ], f32, tag="ov")
        nc.vector.tensor_add(ov[:, :], f2[:, :, 0], f2[:, :, 1])
        nc.gpsimd.dma_start(
            out=tensors["out_dve"][
                :, (2 * g + half) * BQ : (2 * g + half + 1) * BQ
            ],
            in_=ov[:, :],
        )

    # Per-chunk evac: chunk q = psum partition 32q (h in [16q, 16q+16)),
    # copied to SBUF right after the last block's part q, alternating
    # ACT/DVE engines, out-DMA on alternating ACT/sync HWDGE queues.
    CW = 16 * BC
    ots = [
        out_pool.tile([1, CW], f32, tag=f"ot{q}", name=f"ot{q}")
        for q in range(4)
    ]

    def emit_evac(q):
        ot = ots[q]
        c0 = chunk_cols(q)
        pp = chunk_pos(q)
        if q < 3:
            # Mid-stream: single ACT copy + ACT-queue DMA, fully hidden
            # under the next phase's matmuls.
            nc.scalar.activation(
                out=ot[0:1, :],
                in_=ps[pp : pp + 1, c0 : c0 + CW],
                func=mybir.ActivationFunctionType.Copy,
                bias=0.0,
                scale=1.0,
            )
            nc.scalar.dma_start(
                out=tensors["out"][q : q + 1, :], in_=ot[0:1, :]
            )
        else:
            # Terminal chunk: split across ACT and DVE so the tail copy
            # is ~1us, DMAs on separate queues.
            HC = CW // 2
            nc.scalar.activation(
                out=ot[0:1, 0:HC],
                in_=ps[pp : pp + 1, c0 : c0 + HC],
                func=mybir.ActivationFunctionType.Copy,
                bias=0.0,
                scale=1.0,
            )
            nc.scalar.dma_start(
                out=tensors["out"][q : q + 1, 0:HC], in_=ot[0:1, 0:HC]
            )
            nc.vector.tensor_copy(
                ot[0:1, HC:CW],
                ps[pp : pp + 1, c0 + HC : c0 + CW],
            )
            nc.gpsimd.dma_start(
                out=tensors["out"][q : q + 1, HC:CW], in_=ot[0:1, HC:CW]
            )

    # h-phased stream: phase q delivers ALL stocks' h-range [16q, 16q+16)
    # (13 PE quarter-tiles + interleaved DVE half-tiles), accumulating
    # into psum partition 32q, banks alternating by q. Chunk q's evac
    # fires at the phase boundary and hides completely under phase q+1's
    # matmuls (different psum partition AND different banks, so no false
    # hazard). Only chunk 3's evac is in the tail, split across ACT+DVE.
    dve_after = {}
    for k, slot in enumerate(DV_SLOTS):
        dve_after.setdefault(slot, []).append((k // 2, k % 2))

    flat = 0
    for q in range(4):
        for j in range(N_PE):
            K = PE_KS[j]
            et = pes_pool.tile([K, 16 * BC], e3, tag="pes", name=f"pe{q}_{j}")
            nc.sync.dma_start(
                out=et[:, :],
                in_=tensors[f"encpe{j}"][:, q * 16 * BC : (q + 1) * 16 * BC],
            )
            for hh in range(16):
                h = q * 16 + hh
                mm(
                    h,
                    rel16_t[0:K, j * H + h : j * H + h + 1],
                    et[:, hh * BC : (hh + 1) * BC],
                    j == 0,
                    j == N_PE - 1,
                )
            for g, half in dve_after.get(flat, ()):
                emit_dve_half(g, half)
            flat += 1
        emit_evac(q)
        for g, half in dve_after.get(j, ()):
            emit_dve_half(g, half)


def _get_nc():
    key = (
        N_DVE, tuple(PE_KS), WARMUP, WARMUP_N, MID_PARTS, LAST_PARTS,
        tuple(DV_SLOTS), EVAC_INLINE, PSUM_ALT,
    )
    if key in _NC_CACHE:
        return _NC_CACHE[key]
    from contextlib import ExitStack

    bf16 = mybir.dt.bfloat16
    e3 = mybir.dt.float8e3
    nc = bacc.Bacc("TRN2")
    tensors = {}
    for j, K in enumerate(PE_KS):
        tensors[f"encpe{j}"] = nc.dram_tensor(
            f"encpe{j}", [K, H * BC], e3, kind="ExternalInput"
        )
    for g in range(N_DVE):
        tensors[f"encdv{g}"] = nc.dram_tensor(
            f"encdv{g}", [128, BH * K_DVE], e3, kind="ExternalInput"
        )
    tensors["rel16"] = nc.dram_tensor(
        "rel16", [128, N_PE * H], bf16, kind="ExternalInput"
    )
    tensors["rel_dve"] = nc.dram_tensor(
        "rel_dve", [128, N_DVE * K_DVE], bf16, kind="ExternalInput"
    )
    tensors["out"] = nc.dram_tensor(
        "out", [4, 16 * BC], mybir.dt.float32, kind="ExternalOutput"
    )
    tensors["out_dve"] = nc.dram_tensor(
        "out_dve", [128, N_DVE * BH], mybir.dt.float32, kind="ExternalOutput"
    )
    with ExitStack() as ctx:
        tc = ctx.enter_context(tile.TileContext(nc))
        _build(nc, tc, tensors, ctx)
    nc.finalize()
    _NC_CACHE[key] = (nc, tensors)
    return _NC_CACHE[key]


def kernel(stock_idx, encoded_states, relationship_matrix, W1, b1, W2, b2):
    global LAST_RESULT
    idx = int(np.asarray(stock_idx))
    enc = np.asarray(encoded_states, dtype=np.float32)
    relationships = np.asarray(relationship_matrix[idx], dtype=np.float32)  # [S, H]
    W1 = np.asarray(W1, dtype=np.float32)
    W2 = np.asarray(W2, dtype=np.float32)
    b1 = np.asarray(b1, dtype=np.float32)
    b2 = np.asarray(b2, dtype=np.float32)

    # Tiny 2-layer MLP + mask on host (0.006% of FLOPs).
    hmid = np.maximum(relationships @ W1.T + b1, 0.0)
    rel_enc = (hmid @ W2.T + b2).astype(np.float32)  # [S, H]
    rel_enc[idx, :] = 0.0

    # Stationary layouts (shared by all cores).
    rel16 = np.zeros((128, N_PE * H), np.float32)
    for j, (K, off) in enumerate(zip(PE_KS, PE_OFFS)):
        rel16[0:K, j * H : (j + 1) * H] = rel_enc[off : off + K, :]
    rel16 = rel16.astype(ml_dtypes.bfloat16)
    rdh = np.ascontiguousarray(rel_enc[S_PE:, :].T)  # [H, N_DVE*K_DVE]
    rel_dve = np.vstack([rdh, rdh]).astype(ml_dtypes.bfloat16)  # [128, ...]

    e3 = ml_dtypes.float8_e3m4
    in_maps = []
    for c in range(N_CORES):
        ec = enc[c * BC : (c + 1) * BC]  # [BC, S, H]
        m = {"rel16": rel16, "rel_dve": rel_dve}
        for j, (K, off) in enumerate(zip(PE_KS, PE_OFFS)):
            # [K, H, BC]
            blk = np.ascontiguousarray(ec[:, off : off + K, :].transpose(1, 2, 0))
            m[f"encpe{j}"] = blk.astype(e3).reshape(K, H * BC)
        dv = ec[:, S_PE:, :].reshape(2, BH, N_DVE, K_DVE, H)
        for g in range(N_DVE):
            # [half, h, b, s] -> [128, BH*K_DVE]
            blk = np.ascontiguousarray(dv[:, :, g].transpose(0, 3, 1, 2))
            m[f"encdv{g}"] = blk.astype(e3).reshape(128, BH * K_DVE)
        in_maps.append(m)

    if not TRACE:
        os.environ["BASS_NEVER_TRACE"] = "1"
    nc, _ = _get_nc()
    res = run_bass_kernel_spmd(
        nc,
        in_maps,
        core_ids=list(range(N_CORES)),
        trace=TRACE,
        trace_cores=list(range(N_CORES)) if TRACE else None,
    )
    LAST_RESULT = res
    out = np.zeros((B, H), dtype=np.float32)
    for c, r in enumerate(res.results):
        o = np.asarray(r["out"], dtype=np.float32).reshape(4, 16, BC)
        # out[b, h] with h = 16*(psum row q) + col group
        out[c * BC : (c + 1) * BC, :] = o.transpose(2, 0, 1).reshape(BC, H)
        # out_dve cols: (2g+half)*BQ + bq; batch = halfP*64 + half*32 + bq
        odv = np.asarray(r["out_dve"], dtype=np.float32).reshape(128, N_DVE, 2, BH // 2)
        odc = (
            odv.sum(axis=1)
            .reshape(2, H, 2, BH // 2)
            .transpose(0, 2, 3, 1)
            .reshape(BC, H)
        )
        out[c * BC : (c + 1) * BC, :] += odc
    return out


# revision 4
# speedup vs baseline: 1.0072x; 1.0072x over previous
"""Trainium2 Bass kernel for nn_CrossStockRelationship, v2.

Computation (reference):
    rel_encoded = MLP(relationship_matrix[stock_idx])      # [S, H], tiny
    rel_encoded[stock_idx] = 0                             # mask
    out[b, h]  = sum_s encoded_states[b, s, h] * rel_encoded[s, h]

Memory-bound. Device ships every enc element once as fp8 e3m4 (1 B)
-> per-core DMA floor 16.4 MB / 360 B/ns = 45.5 us. Engines split the
contraction so each stays under that window:

- h-PHASED stream: phase q delivers ALL stocks' h-range [16q,16q+16)
  (13 PE quarter-tiles + interleaved DVE half-tiles). Chunk q's PSUM
  accumulation closes at the phase boundary, so its evacuation overlaps
  phase q+1's matmuls; only chunk 3's evac is in the tail.
- PE: 13 blocks (12xK=124 + 1xK=128 = 1616 stocks) as per-h matvecs,
  stationary rel[:, h] bf16, moving enc[s, (h b)] e3m4, f32 PSUM.
  PSUM: chunk q -> partition 32*(q//2) (only 0/32 are HW-encodable),
  banks (q%2)*4..+3, so evac reads and next-phase writes touch
  different banks (the Tile hazard tracker is partition-blind).
  start=True zeroes a whole 2KB psum bank (zero region, 4 h-columns):
  it is issued only on each bank's first h, stop on its last —
  per-h start flags wipe neighbouring h's first-block contributions
  (measured 0.24 rel err, reproduced exactly in CoreSim).
- DVE: 3 blocks of 128 stocks in half-batch granules. ACT converts
  e3m4 -> bf16 (engine otherwise idle), DVE does bf16 mul (2x mode) +
  in-place binary tree reduce, f32 for the last two levels.
- Error: e3m4 quantization of enc gives 1.35e-2 (host-simulated);
  device measures 1.36e-2 total vs the 2e-2 gate. (The baseline's
  extra ~1.15e-2 "device floor" was the zero-region wipe; the per-bank
  start/stop protocol removes it.)
"""

import os
import sys

for _p in ("/opt/trn_rl_repo", "/root/.axon_site/_ro/trn_rl_repo"):
    if os.path.isdir(_p) and _p not in sys.path:
        sys.path.insert(0, _p)

import numpy as np
import ml_dtypes

import concourse.bass as bass
import concourse.bacc as bacc
import concourse.tile as tile
from concourse import mybir
from concourse.bass_utils import run_bass_kernel_spmd

N_CORES = 8
B = 1024
S = 2000
H = 64
BC = B // N_CORES  # 128 batches per core
BH = BC // 2  # 64 batches per DVE partition-half

# PE blocks: 12 x 124 + 1 x 128 = 1616 stocks
N_DVE = int(os.environ.get("KV2_NDVE", "3"))
K_DVE = 128
S_PE = S - N_DVE * K_DVE
_npe_full = S_PE // 124  # blocks of 124
_rem = S_PE - (_npe_full - 1) * 124 if S_PE % 124 else 0
if S_PE % 124 == 0:
    PE_KS = [124] * _npe_full
else:
    # last block absorbs the remainder (<= 128)
    PE_KS = [124] * (S_PE // 124)
    PE_KS[-1] += S_PE % 124
    assert PE_KS[-1] <= 128
N_PE = len(PE_KS)
PE_OFFS = [sum(PE_KS[:j]) for j in range(N_PE)]
assert sum(PE_KS) == S_PE

WARMUP = os.environ.get("KV2_WARMUP", "0") == "1"
WARMUP_N = int(os.environ.get("KV2_WARMUP_N", "24"))
MID_PARTS = int(os.environ.get("KV2_MID_PARTS", "4"))
LAST_PARTS = int(os.environ.get("KV2_LAST_PARTS", "4"))
EVAC_INLINE = os.environ.get("KV2_EVAC_INLINE", "1") == "1"
PSUM_ALT = os.environ.get("KV2_PSUM_ALT", "1") == "1"
# After which flat part index (phase*N_PE + j) each DVE half-tile streams.
DV_SLOTS = [int(x) for x in os.environ.get(
    "KV2_DV_SLOTS", "9,15,21,27,33,40").split(",")]
assert len(DV_SLOTS) == 2 * N_DVE

TRACE = False
LAST_RESULT = None
_NC_CACHE = {}


def _build(nc, tc, tensors, ctx):
    f32 = mybir.dt.float32
    bf16 = mybir.dt.bfloat16
    e3 = mybir.dt.float8e3

    rel_pool = ctx.enter_context(tc.tile_pool(name="rel", bufs=1))
    pe_pool = ctx.enter_context(tc.tile_pool(name="pe", bufs=4))
    pes_pool = ctx.enter_context(tc.tile_pool(name="pes", bufs=14))
    dv8_pool = ctx.enter_context(tc.tile_pool(name="dv8", bufs=2))
    dv16_pool = ctx.enter_context(tc.tile_pool(name="dv16", bufs=2))
    prod_pool = ctx.enter_context(tc.tile_pool(name="prod", bufs=2))
    fv_pool = ctx.enter_context(tc.tile_pool(name="fv", bufs=2))
    ov_pool = ctx.enter_context(tc.tile_pool(name="ov", bufs=2))
    out_pool = ctx.enter_context(tc.tile_pool(name="out", bufs=2))
    psum_pool = ctx.enter_context(tc.tile_pool(name="psum", bufs=1, space="PSUM"))
    if WARMUP:
        wu_pool = ctx.enter_context(tc.tile_pool(name="wu", bufs=1))

    # stationary weights (tiny, land early)
    rel16_t = rel_pool.tile([128, N_PE * H], bf16)
    nc.scalar.dma_start(out=rel16_t[:, :], in_=tensors["rel16"][:, :])
    rel_dve_t = rel_pool.tile([128, N_DVE * K_DVE], bf16)
    nc.scalar.dma_start(out=rel_dve_t[:, :], in_=tensors["rel_dve"][:, :])

    # PSUM: chunk q = h//16 -> partition 32*(q//2) (only offsets 0/32 are
    # HW-encodable), banks (q%2)*4..+3 via an 8KB column offset. Phase q's
    # evac reads banks disjoint from phase q+1's matmul writes, so the
    # boundary evac overlaps the next phase with no false hazard.
    ps = psum_pool.tile([64, 32 * BC], f32)

    def chunk_pos(q):
        return 32 * (q // 2)

    def chunk_cols(q):
        return (q % 2) * 16 * BC

    def mm(h, lhsT, rhs, first, last):
        pos = chunk_pos(h // 16)
        col = chunk_cols(h // 16) + (h % 16) * BC
        # start=True zeroes the whole 2KB psum bank (zero region), which
        # holds 4 h-columns: issue it only for the bank's first h — the
        # other three inherit the pending-zero (read-as-zero) state.
        # Symmetrically, stop only on the bank's last h.
        nc.tensor.matmul(
            out=ps[pos : pos + 1, col : col + BC],
            lhsT=lhsT,
            rhs=rhs,
            start=first and (h % 4 == 0),
            stop=last and (h % 4 == 3),
            # the group checker can't express a 4-subcolumn shared-bank
            # group; correctness is enforced by the start/stop protocol
            skip_group_check=True,
            tile_position=(0, pos),
        )

    if WARMUP:
        # Keep the PE busy (and its clock ramping) from t~0.3us until the
        # first real tile lands: dummy matmuls on a memset tile into a psum
        # region the first real (start=True) matmul resets anyway.
        wt = wu_pool.tile([128, 128], bf16)
        nc.gpsimd.memset(wt[:, :], 0.0)
        for _ in range(WARMUP_N):
            nc.tensor.matmul(
                out=ps[0:1, 0:BC],
                lhsT=wt[:, 0:1],
                rhs=wt[:, 0:BC],
                start=True,
                stop=True,
                skip_group_check=True,
                tile_position=(0, 0),
            )

    # DVE-block pipeline, in batch-half granules (BQ=32 batches x 128
    # stocks): DMA (sync queue, interleaved into the PE tile stream) ->
    # ACT fp8->bf16 convert -> DVE bf16 mul (2x mode) + in-place tree ->
    # out DMA (gpsimd/SWDGE, mid-kernel so its latency is hidden).
    BQ = BH // 2
    ov_all = ov_pool.tile([128, 2 * N_DVE * BQ], f32, name="ov_all")

    def emit_dve_half(g, half):
        et8 = dv8_pool.tile([128, BQ * K_DVE], e3, tag="dv8")
        nc.sync.dma_start(
            out=et8[:, :],
            in_=tensors[f"encdv{g}"][:, half * BQ * K_DVE : (half + 1) * BQ * K_DVE],
        )
        et16 = dv16_pool.tile([128, BQ * K_DVE], bf16, tag="dv16")
        nc.scalar.activation(
            out=et16[:, :],
            in_=et8[:, :],
            func=mybir.ActivationFunctionType.Copy,
            bias=0.0,
            scale=1.0,
        )
        pt = prod_pool.tile([128, BQ * K_DVE], bf16, tag="prod")
        rb = (
            rel_dve_t[:, g * K_DVE : (g + 1) * K_DVE]
            .rearrange("p (o s) -> p o s", o=1)
            .broadcast_to([128, BQ, K_DVE])
        )
        pv = pt[:, :].rearrange("p (b s) -> p b s", s=K_DVE)
        nc.vector.tensor_mul(
            pv, et16[:, :].rearrange("p (b s) -> p b s", s=K_DVE), rb
        )
        s_len = K_DVE
        while s_len > 4:
            nc.vector.tensor_add(
                pv[:, :, 0 : s_len // 2],
                pv[:, :, 0 : s_len // 2],
                pv[:, :, s_len // 2 : s_len],
            )
            s_len //= 2
        fv = fv_pool.tile([128, BQ * 2], f32, tag="fv")
        f2 = fv[:, :].rearrange("p (b s) -> p b s", s=2)
        nc.vector.tensor_add(f2[:, :, :], pv[:, :, 0:2], pv[:, :, 2:4])
        k = 2 * g + half
        nc.vector.tensor_add(
            ov_all[:, k * BQ : (k + 1) * BQ], f2[:, :, 0], f2[:, :, 1]
        )
        if k == 2 * N_DVE - 1:
            # all six half-results staged in one tile -> one DMA (>=512B
            # per partition, so no small-descriptor penalty)
            nc.gpsimd.dma_start(
                out=tensors["out_dve"][:, :], in_=ov_all[:, :]
            )

    # Per-chunk evac: chunk q = psum partition 32q (h in [16q, 16q+16)),
    # copied to SBUF right after the last block's part q, alternating
    # ACT/DVE engines, out-DMA on alternating ACT/sync HWDGE queues.
    CW = 16 * BC
    ots = [
        out_pool.tile([1, CW], f32, tag=f"ot{q}", name=f"ot{q}")
        for q in range(4)
    ]

    def emit_evac(q):
        ot = ots[q]
        c0 = chunk_cols(q)
        pp = chunk_pos(q)
        if q < 3:
            # Mid-stream: single ACT copy + ACT-queue DMA, fully hidden
            # under the next phase's matmuls.
            nc.scalar.activation(
                out=ot[0:1, :],
                in_=ps[pp : pp + 1, c0 : c0 + CW],
                func=mybir.ActivationFunctionType.Copy,
                bias=0.0,
                scale=1.0,
            )
            nc.scalar.dma_start(
                out=tensors["out"][q : q + 1, :], in_=ot[0:1, :]
            )
        else:
            # Terminal chunk: split across ACT and DVE so the tail copy
            # is ~1us, DMAs on separate queues.
            HC = CW // 2
            nc.scalar.activation(
                out=ot[0:1, 0:HC],
                in_=ps[pp : pp + 1, c0 : c0 + HC],
                func=mybir.ActivationFunctionType.Copy,
                bias=0.0,
                scale=1.0,
            )
            nc.scalar.dma_start(
                out=tensors["out"][q : q + 1, 0:HC], in_=ot[0:1, 0:HC]
            )
            nc.vector.tensor_copy(
                ot[0:1, HC:CW],
                ps[pp : pp + 1, c0 + HC : c0 + CW],
            )
            nc.gpsimd.dma_start(
                out=tensors["out"][q : q + 1, HC:CW], in_=ot[0:1, HC:CW]
            )

    # h-phased stream: phase q delivers ALL stocks' h-range [16q, 16q+16)
    # (13 PE quarter-tiles + interleaved DVE half-tiles), accumulating
    # into psum partition 32q, banks alternating by q. Chunk q's evac
    # fires at the phase boundary and hides completely under phase q+1's
    # matmuls (different psum partition AND different banks, so no false
    # hazard). Only chunk 3's evac is in the tail, split across ACT+DVE.
    dve_after = {}
    for k, slot in enumerate(DV_SLOTS):
        dve_after.setdefault(slot, []).append((k // 2, k % 2))

    flat = 0
    for q in range(4):
        for j in range(N_PE):
            K = PE_KS[j]
            et = pes_pool.tile([K, 16 * BC], e3, tag="pes", name=f"pe{q}_{j}")
            nc.sync.dma_start(
                out=et[:, :],
                in_=tensors[f"encpe{j}"][:, q * 16 * BC : (q + 1) * 16 * BC],
            )
            for hh in range(16):
                h = q * 16 + hh
                mm(
                    h,
                    rel16_t[0:K, j * H + h : j * H + h + 1],
                    et[:, hh * BC : (hh + 1) * BC],
                    j == 0,
                    j == N_PE - 1,
                )
            for g, half in dve_after.get(flat, ()):
                emit_dve_half(g, half)
            flat += 1
        emit_evac(q)
        for g, half in dve_after.get(j, ()):
            emit_dve_half(g, half)


def _get_nc():
    key = (
        N_DVE, tuple(PE_KS), WARMUP, WARMUP_N, MID_PARTS, LAST_PARTS,
        tuple(DV_SLOTS), EVAC_INLINE, PSUM_ALT,
    )
    if key in _NC_CACHE:
        return _NC_CACHE[key]
    from contextlib import ExitStack

    bf16 = mybir.dt.bfloat16
    e3 = mybir.dt.float8e3
    nc = bacc.Bacc("TRN2")
    tensors = {}
    for j, K in enumerate(PE_KS):
        tensors[f"encpe{j}"] = nc.dram_tensor(
            f"encpe{j}", [K, H * BC], e3, kind="ExternalInput"
        )
    for g in range(N_DVE):
        tensors[f"encdv{g}"] = nc.dram_tensor(
            f"encdv{g}", [128, BH * K_DVE], e3, kind="ExternalInput"
        )
    tensors["rel16"] = nc.dram_tensor(
        "rel16", [128, N_PE * H], bf16, kind="ExternalInput"
    )
    tensors["rel_dve"] = nc.dram_tensor(
        "rel_dve", [128, N_DVE * K_DVE], bf16, kind="ExternalInput"
    )
    tensors["out"] = nc.dram_tensor(
        "out", [4, 16 * BC], mybir.dt.float32, kind="ExternalOutput"
    )
    tensors["out_dve"] = nc.dram_tensor(
        "out_dve", [128, N_DVE * BH], mybir.dt.float32, kind="ExternalOutput"
    )
    with ExitStack() as ctx:
        tc = ctx.enter_context(tile.TileContext(nc))
        _build(nc, tc, tensors, ctx)
    nc.finalize()
    _NC_CACHE[key] = (nc, tensors)
    return _NC_CACHE[key]


def kernel(stock_idx, encoded_states, relationship_matrix, W1, b1, W2, b2):
    global LAST_RESULT
    idx = int(np.asarray(stock_idx))
    enc = np.asarray(encoded_states, dtype=np.float32)
    relationships = np.asarray(relationship_matrix[idx], dtype=np.float32)  # [S, H]
    W1 = np.asarray(W1, dtype=np.float32)
    W2 = np.asarray(W2, dtype=np.float32)
    b1 = np.asarray(b1, dtype=np.float32)
    b2 = np.asarray(b2, dtype=np.float32)

    # Tiny 2-layer MLP + mask on host (0.006% of FLOPs).
    hmid = np.maximum(relationships @ W1.T + b1, 0.0)
    rel_enc = (hmid @ W2.T + b2).astype(np.float32)  # [S, H]
    rel_enc[idx, :] = 0.0

    # Stationary layouts (shared by all cores).
    rel16 = np.zeros((128, N_PE * H), np.float32)
    for j, (K, off) in enumerate(zip(PE_KS, PE_OFFS)):
        rel16[0:K, j * H : (j + 1) * H] = rel_enc[off : off + K, :]
    rel16 = rel16.astype(ml_dtypes.bfloat16)
    rdh = np.ascontiguousarray(rel_enc[S_PE:, :].T)  # [H, N_DVE*K_DVE]
    rel_dve = np.vstack([rdh, rdh]).astype(ml_dtypes.bfloat16)  # [128, ...]

    e3 = ml_dtypes.float8_e3m4
    in_maps = []
    for c in range(N_CORES):
        ec = enc[c * BC : (c + 1) * BC]  # [BC, S, H]
        m = {"rel16": rel16, "rel_dve": rel_dve}
        for j, (K, off) in enumerate(zip(PE_KS, PE_OFFS)):
            # [K, H, BC]
            blk = np.ascontiguousarray(ec[:, off : off + K, :].transpose(1, 2, 0))
            m[f"encpe{j}"] = blk.astype(e3).reshape(K, H * BC)
        dv = ec[:, S_PE:, :].reshape(2, BH, N_DVE, K_DVE, H)
        for g in range(N_DVE):
            # [half, h, b, s] -> [128, BH*K_DVE]
            blk = np.ascontiguousarray(dv[:, :, g].transpose(0, 3, 1, 2))
            m[f"encdv{g}"] = blk.astype(e3).reshape(128, BH * K_DVE)
        in_maps.append(m)

    if not TRACE:
        os.environ["BASS_NEVER_TRACE"] = "1"
    nc, _ = _get_nc()
    res = run_bass_kernel_spmd(
        nc,
        in_maps,
        core_ids=list(range(N_CORES)),
        trace=TRACE,
        trace_cores=list(range(N_CORES)) if TRACE else None,
    )
    LAST_RESULT = res
    out = np.zeros((B, H), dtype=np.float32)
    for c, r in enumerate(res.results):
        o = np.asarray(r["out"], dtype=np.float32).reshape(4, 16, BC)
        # out[b, h] with h = 16*(psum row q) + col group
        out[c * BC : (c + 1) * BC, :] = o.transpose(2, 0, 1).reshape(BC, H)
        # out_dve cols: (2g+half)*BQ + bq; batch = halfP*64 + half*32 + bq
        odv = np.asarray(r["out_dve"], dtype=np.float32).reshape(128, N_DVE, 2, BH // 2)
        odc = (
            odv.sum(axis=1)
            .reshape(2, H, 2, BH // 2)
            .transpose(0, 2, 3, 1)
            .reshape(BC, H)
        )
        out[c * BC : (c + 1) * BC, :] += odc
    return out


# revision 5
# speedup vs baseline: 1.0163x; 1.0090x over previous
"""Trainium2 Bass kernel for nn_CrossStockRelationship, v2.

Computation (reference):
    rel_encoded = MLP(relationship_matrix[stock_idx])      # [S, H], tiny
    rel_encoded[stock_idx] = 0                             # mask
    out[b, h]  = sum_s encoded_states[b, s, h] * rel_encoded[s, h]

Memory-bound. Device ships every enc element once as fp8 e3m4 (1 B)
-> per-core DMA floor 16.4 MB / 360 B/ns = 45.5 us. Engines split the
contraction so each stays under that window:

- h-PHASED stream: phase q delivers ALL stocks' h-range [16q,16q+16)
  (13 PE quarter-tiles + interleaved DVE half-tiles). Chunk q's PSUM
  accumulation closes at the phase boundary, so its evacuation overlaps
  phase q+1's matmuls; only chunk 3's evac is in the tail.
- PE: 13 blocks (12xK=124 + 1xK=128 = 1616 stocks) as per-h matvecs,
  stationary rel[:, h] bf16, moving enc[s, (h b)] e3m4, f32 PSUM.
  PSUM: chunk q -> partition 32*(q//2) (only 0/32 are HW-encodable),
  banks (q%2)*4..+3, so evac reads and next-phase writes touch
  different banks (the Tile hazard tracker is partition-blind).
  start=True zeroes a whole 2KB psum bank (zero region, 4 h-columns):
  it is issued only on each bank's first h, stop on its last —
  per-h start flags wipe neighbouring h's first-block contributions
  (measured 0.24 rel err, reproduced exactly in CoreSim).
- DVE: 3 blocks of 128 stocks in half-batch granules. ACT converts
  e3m4 -> bf16 (engine otherwise idle), DVE does bf16 mul (2x mode) +
  in-place binary tree reduce, f32 for the last two levels.
- Error: e3m4 quantization of enc gives 1.35e-2 (host-simulated);
  device measures 1.36e-2 total vs the 2e-2 gate. (The baseline's
  extra ~1.15e-2 "device floor" was the zero-region wipe; the per-bank
  start/stop protocol removes it.)
"""

import os
import sys

for _p in ("/opt/trn_rl_repo", "/root/.axon_site/_ro/trn_rl_repo"):
    if os.path.isdir(_p) and _p not in sys.path:
        sys.path.insert(0, _p)

import numpy as np
import ml_dtypes

import concourse.bass as bass
import concourse.bacc as bacc
import concourse.tile as tile
from concourse import mybir
from concourse.bass_utils import run_bass_kernel_spmd

N_CORES = 8
B = 1024
S = 2000
H = 64
BC = B // N_CORES  # 128 batches per core
BH = BC // 2  # 64 batches per DVE partition-half

# PE blocks: 12 x 124 + 1 x 128 = 1616 stocks
N_DVE = int(os.environ.get("KV2_NDVE", "3"))
K_DVE = 128
S_PE = S - N_DVE * K_DVE
_npe_full = S_PE // 124  # blocks of 124
_rem = S_PE - (_npe_full - 1) * 124 if S_PE % 124 else 0
if S_PE % 124 == 0:
    PE_KS = [124] * _npe_full
else:
    # last block absorbs the remainder (<= 128)
    PE_KS = [124] * (S_PE // 124)
    PE_KS[-1] += S_PE % 124
    assert PE_KS[-1] <= 128
N_PE = len(PE_KS)
PE_OFFS = [sum(PE_KS[:j]) for j in range(N_PE)]
assert sum(PE_KS) == S_PE

WARMUP = os.environ.get("KV2_WARMUP", "0") == "1"
WARMUP_N = int(os.environ.get("KV2_WARMUP_N", "24"))
MID_PARTS = int(os.environ.get("KV2_MID_PARTS", "4"))
LAST_PARTS = int(os.environ.get("KV2_LAST_PARTS", "4"))
EVAC_INLINE = os.environ.get("KV2_EVAC_INLINE", "1") == "1"
PSUM_ALT = os.environ.get("KV2_PSUM_ALT", "1") == "1"
# After which flat part index (phase*N_PE + j) each DVE half-tile streams.
DV_SLOTS = [int(x) for x in os.environ.get(
    "KV2_DV_SLOTS", "9,15,21,27,33,40").split(",")]
assert len(DV_SLOTS) == 2 * N_DVE

TRACE = False
LAST_RESULT = None
_NC_CACHE = {}


def _build(nc, tc, tensors, ctx):
    f32 = mybir.dt.float32
    bf16 = mybir.dt.bfloat16
    e3 = mybir.dt.float8e3

    rel_pool = ctx.enter_context(tc.tile_pool(name="rel", bufs=1))
    pe_pool = ctx.enter_context(tc.tile_pool(name="pe", bufs=4))
    pes_pool = ctx.enter_context(tc.tile_pool(name="pes", bufs=14))
    dv8_pool = ctx.enter_context(tc.tile_pool(name="dv8", bufs=2))
    dv16_pool = ctx.enter_context(tc.tile_pool(name="dv16", bufs=2))
    prod_pool = ctx.enter_context(tc.tile_pool(name="prod", bufs=2))
    fv_pool = ctx.enter_context(tc.tile_pool(name="fv", bufs=2))
    ov_pool = ctx.enter_context(tc.tile_pool(name="ov", bufs=2))
    out_pool = ctx.enter_context(tc.tile_pool(name="out", bufs=2))
    psum_pool = ctx.enter_context(tc.tile_pool(name="psum", bufs=1, space="PSUM"))
    if WARMUP:
        wu_pool = ctx.enter_context(tc.tile_pool(name="wu", bufs=1))

    # stationary weights (tiny, land early)
    rel16_t = rel_pool.tile([128, N_PE * H], bf16)
    nc.scalar.dma_start(out=rel16_t[:, :], in_=tensors["rel16"][:, :])
    rel_dve_t = rel_pool.tile([128, N_DVE * K_DVE], bf16)
    nc.scalar.dma_start(out=rel_dve_t[:, :], in_=tensors["rel_dve"][:, :])

    # PSUM: chunk q = h//16 -> partition 32*(q//2) (only offsets 0/32 are
    # HW-encodable), banks (q%2)*4..+3 via an 8KB column offset. Phase q's
    # evac reads banks disjoint from phase q+1's matmul writes, so the
    # boundary evac overlaps the next phase with no false hazard.
    ps = psum_pool.tile([64, 32 * BC], f32)

    def chunk_pos(q):
        return 32 * (q // 2)

    def chunk_cols(q):
        return (q % 2) * 16 * BC

    def mm(h, lhsT, rhs, first, last):
        pos = chunk_pos(h // 16)
        col = chunk_cols(h // 16) + (h % 16) * BC
        # start=True zeroes the whole 2KB psum bank (zero region), which
        # holds 4 h-columns: issue it only for the bank's first h — the
        # other three inherit the pending-zero (read-as-zero) state.
        # Symmetrically, stop only on the bank's last h.
        nc.tensor.matmul(
            out=ps[pos : pos + 1, col : col + BC],
            lhsT=lhsT,
            rhs=rhs,
            start=first and (h % 4 == 0),
            stop=last and (h % 4 == 3),
            # the group checker can't express a 4-subcolumn shared-bank
            # group; correctness is enforced by the start/stop protocol
            skip_group_check=True,
            tile_position=(0, pos),
        )

    if WARMUP:
        # Keep the PE busy (and its clock ramping) from t~0.3us until the
        # first real tile lands: dummy matmuls on a memset tile into a psum
        # region the first real (start=True) matmul resets anyway.
        wt = wu_pool.tile([128, 128], bf16)
        nc.gpsimd.memset(wt[:, :], 0.0)
        for _ in range(WARMUP_N):
            nc.tensor.matmul(
                out=ps[0:1, 0:BC],
                lhsT=wt[:, 0:1],
                rhs=wt[:, 0:BC],
                start=True,
                stop=True,
                skip_group_check=True,
                tile_position=(0, 0),
            )

    # DVE-block pipeline, in batch-half granules (BQ=32 batches x 128
    # stocks): DMA (sync queue, interleaved into the PE tile stream) ->
    # ACT fp8->bf16 convert -> DVE bf16 mul (2x mode) + in-place tree ->
    # out DMA (gpsimd/SWDGE, mid-kernel so its latency is hidden).
    BQ = BH // 2
    ov_all = ov_pool.tile([128, 2 * N_DVE * BQ], f32, name="ov_all")

    def emit_dve_half(g, half):
        et8 = dv8_pool.tile([128, BQ * K_DVE], e3, tag="dv8")
        nc.sync.dma_start(
            out=et8[:, :],
            in_=tensors[f"encdv{g}"][:, half * BQ * K_DVE : (half + 1) * BQ * K_DVE],
        )
        et16 = dv16_pool.tile([128, BQ * K_DVE], bf16, tag="dv16")
        nc.scalar.activation(
            out=et16[:, :],
            in_=et8[:, :],
            func=mybir.ActivationFunctionType.Copy,
            bias=0.0,
            scale=1.0,
        )
        pt = prod_pool.tile([128, BQ * K_DVE], bf16, tag="prod")
        rb = (
            rel_dve_t[:, g * K_DVE : (g + 1) * K_DVE]
            .rearrange("p (o s) -> p o s", o=1)
            .broadcast_to([128, BQ, K_DVE])
        )
        pv = pt[:, :].rearrange("p (b s) -> p b s", s=K_DVE)
        nc.vector.tensor_mul(
            pv, et16[:, :].rearrange("p (b s) -> p b s", s=K_DVE), rb
        )
        s_len = K_DVE
        while s_len > 4:
            nc.vector.tensor_add(
                pv[:, :, 0 : s_len // 2],
                pv[:, :, 0 : s_len // 2],
                pv[:, :, s_len // 2 : s_len],
            )
            s_len //= 2
        fv = fv_pool.tile([128, BQ * 2], f32, tag="fv")
        f2 = fv[:, :].rearrange("p (b s) -> p b s", s=2)
        nc.vector.tensor_add(f2[:, :, :], pv[:, :, 0:2], pv[:, :, 2:4])
        k = 2 * g + half
        nc.vector.tensor_add(
            ov_all[:, k * BQ : (k + 1) * BQ], f2[:, :, 0], f2[:, :, 1]
        )
        if k == 2 * N_DVE - 1:
            # all six half-results staged in one tile -> one DMA (>=512B
            # per partition, so no small-descriptor penalty)
            nc.gpsimd.dma_start(
                out=tensors["out_dve"][:, :], in_=ov_all[:, :]
            )

    # Per-chunk evac: chunk q = psum partition 32q (h in [16q, 16q+16)),
    # copied to SBUF right after the last block's part q, alternating
    # ACT/DVE engines, out-DMA on alternating ACT/sync HWDGE queues.
    CW = 16 * BC
    ots = [
        out_pool.tile([1, CW], f32, tag=f"ot{q}", name=f"ot{q}")
        for q in range(4)
    ]

    def emit_evac(q):
        ot = ots[q]
        c0 = chunk_cols(q)
        pp = chunk_pos(q)
        if q < 3:
            # Mid-stream: single ACT copy + ACT-queue DMA, fully hidden
            # under the next phase's matmuls.
            nc.scalar.activation(
                out=ot[0:1, :],
                in_=ps[pp : pp + 1, c0 : c0 + CW],
                func=mybir.ActivationFunctionType.Copy,
                bias=0.0,
                scale=1.0,
            )
            nc.scalar.dma_start(
                out=tensors["out"][q : q + 1, :], in_=ot[0:1, :]
            )
        else:
            # Terminal chunk: split across ACT and DVE so the tail copy
            # is ~1us, DMAs on separate queues.
            HC = CW // 2
            nc.scalar.activation(
                out=ot[0:1, 0:HC],
                in_=ps[pp : pp + 1, c0 : c0 + HC],
                func=mybir.ActivationFunctionType.Copy,
                bias=0.0,
                scale=1.0,
            )
            nc.vector.tensor_copy(
                ot[0:1, HC:CW],
                ps[pp : pp + 1, c0 + HC : c0 + CW],
            )
            # one combined DMA after both copies: a single HWDGE chain and
            # a single completion semaphore end ~0.75us sooner than two
            # contending per-half chains
            nc.sync.dma_start(
                out=tensors["out"][q : q + 1, :], in_=ot[0:1, :]
            )

    # h-phased stream: phase q delivers ALL stocks' h-range [16q, 16q+16)
    # (13 PE quarter-tiles + interleaved DVE half-tiles), accumulating
    # into psum partition 32q, banks alternating by q. Chunk q's evac
    # fires at the phase boundary and hides completely under phase q+1's
    # matmuls (different psum partition AND different banks, so no false
    # hazard). Only chunk 3's evac is in the tail, split across ACT+DVE.
    dve_after = {}
    for k, slot in enumerate(DV_SLOTS):
        dve_after.setdefault(slot, []).append((k // 2, k % 2))

    flat = 0
    for q in range(4):
        for j in range(N_PE):
            K = PE_KS[j]
            et = pes_pool.tile([K, 16 * BC], e3, tag="pes", name=f"pe{q}_{j}")
            nc.sync.dma_start(
                out=et[:, :],
                in_=tensors[f"encpe{j}"][:, q * 16 * BC : (q + 1) * 16 * BC],
            )
            for hh in range(16):
                h = q * 16 + hh
                mm(
                    h,
                    rel16_t[0:K, j * H + h : j * H + h + 1],
                    et[:, hh * BC : (hh + 1) * BC],
                    j == 0,
                    j == N_PE - 1,
                )
            for g, half in dve_after.get(flat, ()):
                emit_dve_half(g, half)
            flat += 1
        emit_evac(q)
        for g, half in dve_after.get(j, ()):
            emit_dve_half(g, half)


def _get_nc():
    key = (
        N_DVE, tuple(PE_KS), WARMUP, WARMUP_N, MID_PARTS, LAST_PARTS,
        tuple(DV_SLOTS), EVAC_INLINE, PSUM_ALT,
    )
    if key in _NC_CACHE:
        return _NC_CACHE[key]
    from contextlib import ExitStack

    bf16 = mybir.dt.bfloat16
    e3 = mybir.dt.float8e3
    nc = bacc.Bacc("TRN2")
    tensors = {}
    for j, K in enumerate(PE_KS):
        tensors[f"encpe{j}"] = nc.dram_tensor(
            f"encpe{j}", [K, H * BC], e3, kind="ExternalInput"
        )
    for g in range(N_DVE):
        tensors[f"encdv{g}"] = nc.dram_tensor(
            f"encdv{g}", [128, BH * K_DVE], e3, kind="ExternalInput"
        )
    tensors["rel16"] = nc.dram_tensor(
        "rel16", [128, N_PE * H], bf16, kind="ExternalInput"
    )
    tensors["rel_dve"] = nc.dram_tensor(
        "rel_dve", [128, N_DVE * K_DVE], bf16, kind="ExternalInput"
    )
    tensors["out"] = nc.dram_tensor(
        "out", [4, 16 * BC], mybir.dt.float32, kind="ExternalOutput"
    )
    tensors["out_dve"] = nc.dram_tensor(
        "out_dve", [128, N_DVE * BH], mybir.dt.float32, kind="ExternalOutput"
    )
    with ExitStack() as ctx:
        tc = ctx.enter_context(tile.TileContext(nc))
        _build(nc, tc, tensors, ctx)
    nc.finalize()
    _NC_CACHE[key] = (nc, tensors)
    return _NC_CACHE[key]


def kernel(stock_idx, encoded_states, relationship_matrix, W1, b1, W2, b2):
    global LAST_RESULT
    idx = int(np.asarray(stock_idx))
    enc = np.asarray(encoded_states, dtype=np.float32)
    relationships = np.asarray(relationship_matrix[idx], dtype=np.float32)  # [S, H]
    W1 = np.asarray(W1, dtype=np.float32)
    W2 = np.asarray(W2, dtype=np.float32)
    b1 = np.asarray(b1, dtype=np.float32)
    b2 = np.asarray(b2, dtype=np.float32)

    # Tiny 2-layer MLP + mask on host (0.006% of FLOPs).
    hmid = np.maximum(relationships @ W1.T + b1, 0.0)
    rel_enc = (hmid @ W2.T + b2).astype(np.float32)  # [S, H]
    rel_enc[idx, :] = 0.0

    # Stationary layouts (shared by all cores).
    rel16 = np.zeros((128, N_PE * H), np.float32)
    for j, (K, off) in enumerate(zip(PE_KS, PE_OFFS)):
        rel16[0:K, j * H : (j + 1) * H] = rel_enc[off : off + K, :]
    rel16 = rel16.astype(ml_dtypes.bfloat16)
    rdh = np.ascontiguousarray(rel_enc[S_PE:, :].T)  # [H, N_DVE*K_DVE]
    rel_dve = np.vstack([rdh, rdh]).astype(ml_dtypes.bfloat16)  # [128, ...]

    e3 = ml_dtypes.float8_e3m4
    in_maps = []
    for c in range(N_CORES):
        ec = enc[c * BC : (c + 1) * BC]  # [BC, S, H]
        m = {"rel16": rel16, "rel_dve": rel_dve}
        for j, (K, off) in enumerate(zip(PE_KS, PE_OFFS)):
            # [K, H, BC]
            blk = np.ascontiguousarray(ec[:, off : off + K, :].transpose(1, 2, 0))
            m[f"encpe{j}"] = blk.astype(e3).reshape(K, H * BC)
        dv = ec[:, S_PE:, :].reshape(2, BH, N_DVE, K_DVE, H)
        for g in range(N_DVE):
            # [half, h, b, s] -> [128, BH*K_DVE]
            blk = np.ascontiguousarray(dv[:, :, g].transpose(0, 3, 1, 2))
            m[f"encdv{g}"] = blk.astype(e3).reshape(128, BH * K_DVE)
        in_maps.append(m)

    if not TRACE:
        os.environ["BASS_NEVER_TRACE"] = "1"
    nc, _ = _get_nc()
    res = run_bass_kernel_spmd(
        nc,
        in_maps,
        core_ids=list(range(N_CORES)),
        trace=TRACE,
        trace_cores=list(range(N_CORES)) if TRACE else None,
    )
    LAST_RESULT = res
    out = np.zeros((B, H), dtype=np.float32)
    for c, r in enumerate(res.results):
        o = np.asarray(r["out"], dtype=np.float32).reshape(4, 16, BC)
        # out[b, h] with h = 16*(psum row q) + col group
        out[c * BC : (c + 1) * BC, :] = o.transpose(2, 0, 1).reshape(BC, H)
        # out_dve cols: (2g+half)*BQ + bq; batch = halfP*64 + half*32 + bq
        odv = np.asarray(r["out_dve"], dtype=np.float32).reshape(128, N_DVE, 2, BH // 2)
        odc = (
            odv.sum(axis=1)
            .reshape(2, H, 2, BH // 2)
            .transpose(0, 2, 3, 1)
            .reshape(BC, H)
        )
        out[c * BC : (c + 1) * BC, :] += odc
    return out
